# revision 54
# baseline (speedup 1.0000x reference)
"""Causal self-attention (B=1, T=4096, C=768, H=12, D=64) on 8 TRN2 NeuronCores.

Sharding: 8 cores = 4 head-groups (3 heads each) x 2 sequence-groups.
Core c: heads [3*hg, 3*hg+2] where hg=c//2; handles q-chunks of 256 rows,
global chunk g = 2*j + s (s=c%2, j=0..7) -- interleaving balances the causal
triangle so every core runs an identical instruction stream (SPMD), with the
boundary masks supplied as per-core data.

All matmuls fp16 single-pass (fp32 PSUM accumulation).  Flash-style
attention in S^T = K@Q^T orientation; the softmax denominator arrives free
as the ones-column row of (V' P^T).  Causality = block skipping + 0/1
boundary masks.  Every matmul keeps 128-partition operands (diag-packed
K^T, doubled Q, head-paired projection): the PE activity monitor throttles
the clock to 1.2 GHz when it sees half-idle arrays, so full-width operands
are a 2x clock win, not a FLOP nicety.

Perf structure: all inputs DMA'd up front in single transfers; phase-1 V
transposes software-pipelined one matmul-group behind; per-slot tail does
den -> ACT ln -> ACT exp(-x) (reciprocal off the Vector engine) hidden
behind next-slot Q matmuls, then the head-paired projection.
Host sums the 4 head-group partial projections.
"""
import numpy as np

T, C, H, D = 4096, 768, 12, 64
NH = 3          # heads per core
QC = 256        # q rows per slot
P = 128

_nc_cache = {}


def split_multi_waits(nc):
    """Walrus here accepts only one sync wait per instruction: hoist extras
    onto standalone InstEventSemaphore instructions on the same engine."""
    import concourse.mybir as mybir
    n_split = 0
    for f in nc.m.functions:
        for bb in f.blocks:
            new_insts = []
            for inst in bb.instructions:
                si = inst.sync_info
                if si is not None and len(si.on_wait) > 1:
                    for w in si.on_wait[:-1]:
                        nop = mybir.InstEventSemaphore(
                            name=nc.get_next_instruction_name(), ins=[], outs=[])
                        nop.engine = inst.engine
                        nop.sync_info = mybir.SyncInfo(on_wait=[w], on_update=[])
                        nc.register_instruction(nop)
                        new_insts.append(nop)
                        n_split += 1
                    si.on_wait = si.on_wait[-1:]
                new_insts.append(inst)
            bb.instructions[:] = new_insts
    return n_split


def build_nc(Tloc=T):
    import concourse.bass as bass
    import concourse.mybir as mybir
    import concourse.tile as tile
    from concourse.masks import make_identity
    from contextlib import ExitStack

    f32r = mybir.dt.float32r
    f32 = mybir.dt.float32
    f16 = mybir.dt.float16
    EXP = mybir.ActivationFunctionType.Exp
    LN = mybir.ActivationFunctionType.Ln
    CPY = mybir.ActivationFunctionType.Copy
    ADD = mybir.AluOpType.add

    nslot = Tloc // (2 * QC)     # q-chunks per core
    nkb = Tloc // P              # k 128-blocks
    tq = nslot * QC              # q rows per core

    nc = bass.Bass(trn_type="TRN2")
    xt16 = nc.dram_tensor("xt16", [P, 6, Tloc], f16, kind="ExternalInput")
    xtq16 = nc.dram_tensor("xtq16", [P, 6, tq], f16, kind="ExternalInput")
    wkv16 = nc.dram_tensor("wkv16", [P, 6, 6 * D], f16, kind="ExternalInput")
    wq16 = nc.dram_tensor("wq16", [P, 6, 2 * NH * D], f16, kind="ExternalInput")
    wpj16 = nc.dram_tensor("wpj16", [P, 2, C], f16, kind="ExternalInput")
    bias = nc.dram_tensor("bias", [P, 6], f32, kind="ExternalInput")
    mask = nc.dram_tensor("mask", [P, nslot, 1024], f16, kind="ExternalInput")
    out = nc.dram_tensor("out", [tq, C], f32, kind="ExternalOutput")

    # m-chunk -> (kind, head) for the packed [k0 k1 k2 v0 v1 v2] KV weights
    kv_map = [[("K", 0), ("K", 1)], [("K", 2), ("V", 0)], [("V", 1), ("V", 2)]]

    with tile.TileContext(nc) as tc, ExitStack() as ctx:
        singles = ctx.enter_context(tc.tile_pool(name="singles", bufs=1))
        ysp = ctx.enter_context(tc.tile_pool(name="ys", bufs=2))
        xthp = ctx.enter_context(tc.tile_pool(name="xth", bufs=2))
        vstp = ctx.enter_context(tc.tile_pool(name="vst", bufs=2))
        qtp = ctx.enter_context(tc.tile_pool(name="qt", bufs=2))
        ptp = ctx.enter_context(tc.tile_pool(name="pt", bufs=3))
        rbp = ctx.enter_context(tc.tile_pool(name="rb", bufs=2))
        ytp = ctx.enter_context(tc.tile_pool(name="yt", bufs=2))
        ostp = ctx.enter_context(tc.tile_pool(name="ost", bufs=2))
        psg = ctx.enter_context(tc.tile_pool(name="psg", bufs=2, space="PSUM"))
        psy = ctx.enter_context(tc.tile_pool(name="psy", bufs=1, space="PSUM"))
        psm = ctx.enter_context(tc.tile_pool(name="psm", bufs=2, space="PSUM"))

        # --- primary weights first so phase 1 can start the moment x lands
        wkv_t = singles.tile([P, 6, 6 * D], f16)
        nc.sync.dma_start(wkv_t, wkv16[:, :, :])
        b_t = singles.tile([P, 6], f32)
        nc.sync.dma_start(b_t, bias[:, :])

        ident16 = singles.tile([P, P], f16)
        make_identity(nc, ident16)
        ones_f = singles.tile([1, 64], f32)
        nc.vector.memset(ones_f, 1.0)
        ones64 = singles.tile([1, 64], f32r)
        nc.vector.tensor_copy(ones64, ones_f)
        onesk_f = singles.tile([P, 32], f16)
        nc.vector.memset(onesk_f, 1.0)

        kt_t = [singles.tile([P, Tloc], f16, tag=f"kt{h}", name=f"kt{h}")
                for h in range(NH)]
        # bottom partition half starts zeroed; odd 64-col halves get K^T rows
        # DMA'd down, then the top copies of those columns are zeroed, giving
        # diag(K^T even-half, K^T odd-half) per 128-col block -> K=128 matmuls
        for h in range(NH):
            nc.vector.memset(kt_t[h][64:P, :], 0.0)
        vp_t = [singles.tile([P, nkb, 65], f16, tag=f"vp{h}", name=f"vp{h}")
                for h in range(NH)]
        for h in range(NH):
            nc.vector.tensor_copy(vp_t[h][:, :, 64], onesk_f[:, :nkb])

        # ---- Phase 1: K^T, V-transposed from x^T, in 4 column-quarters ----
        QT4 = Tloc // 4
        pending = []            # deferred V transposes: (m, g0, vst)

        def flush_pending():
            for (m, g0, vst) in pending:
                for i in range(4):
                    tp = psm.tile([P, P], f16, tag="psm", name="tp")
                    nc.tensor.transpose(tp, vst[:, P * i:P * (i + 1)], ident16)
                    blk = g0 // P + i
                    for sub, (kind, h) in enumerate(kv_map[m]):
                        if kind == "V":
                            nc.scalar.activation(
                                vp_t[h][:, blk, 0:64],
                                tp[:, 64 * sub:64 * sub + 64], CPY)
            pending.clear()

        def mm_chunk(quarter, m, n0, xh_box):
            xh = xh_box[0]
            ps = psg.tile([P, 1024], f32, tag="sg", name="ps")[:, :512]
            for c in range(6):
                nc.tensor.matmul(ps, wkv_t[:, c, P * m:P * m + P],
                                 xh[:, c, n0:n0 + 512],
                                 start=(c == 0), stop=(c == 5))
            flush_pending()
            g0 = QT4 * quarter + n0
            has_v = any(kind == "V" for kind, _ in kv_map[m])
            if has_v:
                # full-block bias-add (per-partition bias covers both
                # 64-row subs); K rows are also staged but unused
                vst = vstp.tile([P, 512], f16, tag="vst", name="vst")
                nc.vector.tensor_scalar(vst, ps, b_t[:, m:m + 1], None, ADD)
                pending.append((m, g0, vst))
            for sub, (kind, h) in enumerate(kv_map[m]):
                if kind == "K":
                    rows = slice(64 * sub, 64 * sub + 64)
                    nc.vector.tensor_scalar(
                        kt_t[h][0:64, g0:g0 + 512], ps[rows],
                        b_t[rows, m:m + 1], None, ADD)

        def kt_fixup(b0, nb=8):
            # diag-pack fixup for k-blocks [b0, b0+nb), so the slots that
            # only need kv up to here can start immediately
            flush_pending()
            for h in range(NH):
                ot = kt_t[h][0:64].rearrange(
                    "p (b t) -> p b t", t=P)[:, b0:b0 + nb, 64:P]
                ob = kt_t[h][64:P].rearrange(
                    "p (b t) -> p b t", t=P)[:, b0:b0 + nb, 64:P]
                nc.sync.dma_start(ob, ot)
                nc.vector.memset(ot, 0.0)

        def make_quarter_work(quarter):
            """Quarter q as a list of small closures, drained one per
            attention group of the two preceding slots."""
            xh_box = [None]

            def dma():
                xh = xthp.tile([P, 6, QT4], f16, tag="xh", name="xh")
                nc.sync.dma_start(
                    xh, xt16[:, :, QT4 * quarter:QT4 * (quarter + 1)])
                xh_box[0] = xh
            items = [dma]
            for m in range(3):
                for n0 in (0, 512):
                    items.append(
                        lambda m=m, n0=n0: mm_chunk(quarter, m, n0, xh_box))
            items.append(lambda: kt_fixup(8 * quarter))
            return items

        # quarter 0 runs up front (slots 0-1 need it immediately)
        xh0_box = [None]
        xh0 = xthp.tile([P, 6, QT4], f16, tag="xh", name="xh")
        nc.sync.dma_start(xh0, xt16[:, :, 0:QT4])
        xh0_box[0] = xh0
        wq_t = singles.tile([P, 6, 2 * NH * D], f16)
        nc.sync.dma_start(wq_t, wq16[:, :, :])
        xtq_t = singles.tile([P, 6, tq], f16)
        nc.sync.dma_start(xtq_t, xtq16[:, :, :])
        mask_t = singles.tile([P, nslot, 1024], f16)
        nc.sync.dma_start(mask_t, mask[:, :, :])
        wpj_t = singles.tile([P, 2, C], f16)
        nc.sync.dma_start(wpj_t, wpj16[:, :, :])
        # slot 0 only needs k-blocks 0-3: run just the first half of
        # quarter 0 up front, defer the second half into slot 0's drains
        for m in range(3):
            mm_chunk(0, m, 0, xh0_box)
        kt_fixup(0, 4)

        # deferred phase-1 chunks (quarter-0 second half now; quarters 1-3
        # and deferred projections join later)
        work_q = [lambda m=m: mm_chunk(0, m, 512, xh0_box) for m in range(3)]
        work_q.append(lambda: kt_fixup(4, 4))
        late_q = []     # previous slot's normalize/projection chunks
        qh_q = []       # next slot's Q-head chunks

        # ---- Phase 2: per q-slot: attention, then (next Q | recip | proj) ----
        qt_cur = [None] * NH

        def q_head(j, h):
            psq = psm.tile([P, QC], f32, tag="psm", name="psq")
            for c in range(6):
                nc.tensor.matmul(psq, wq_t[:, c, P * h:P * (h + 1)],
                                 xtq_t[:, c, QC * j:QC * (j + 1)],
                                 start=(c == 0), stop=(c == 5))
            qh = qtp.tile([P, QC], f16, tag=f"qt{h}", name=f"qt{h}")
            nc.vector.tensor_scalar(qh, psq, b_t[:, 3 + h:4 + h], None, ADD)
            qt_cur[h] = qh

        def q_block(j):
            for h in range(NH):
                q_head(j, h)

        q_block(0)

        def do_slot(j):
            yacc = psy.tile([65, NH * QC], f32, tag="yacc", name="yacc")
            den3 = rbp.tile([1, NH * QC], f32, tag="den", name="den")
            # snapshot this slot's Q tiles BEFORE enqueueing next-slot Q
            # heads, which overwrite qt_cur when drained mid-slot
            my_qt = list(qt_cur)
            # next slot's Q heads drain through this slot's exp-wait gaps so
            # the tail carries no PE work at all
            if j + 1 < nslot:
                qh_q.extend(
                    [lambda h=h: q_head(j + 1, h) for h in range(NH)])

            for h in range(NH):
                yh = yacc[:, QC * h:QC * (h + 1)]

                def s_group(g):
                    sg = psg.tile([P, 1024], f32, tag="sg", name="sg")
                    for i in range(4):
                        kb = 4 * g + i
                        nc.tensor.matmul(sg[:, QC * i:QC * (i + 1)],
                                         kt_t[h][:, P * kb:P * (kb + 1)],
                                         my_qt[h], start=True, stop=True)
                    return sg

                # software pipeline: issue S(g+1) before PV(g) so the PE has
                # work while ACT runs exp(g)
                sg_cur = s_group(0)
                for g in range(j + 1):
                    # drain BEFORE issuing sg_next: a pending kt fixup must
                    # be emitted before any S-group that reads its blocks
                    if work_q:
                        work_q.pop(0)()
                    elif qh_q:
                        qh_q.pop(0)()
                    sg_next = s_group(g + 1) if g < j else None
                    pt = ptp.tile([P, 1024], f16, tag="pt", name="pt")
                    nc.scalar.activation(pt, sg_cur, EXP, scale=0.125)
                    # previous slot's normalize/project chunks drain AFTER
                    # this exp is emitted, so their Ln/Exp land behind it in
                    # the ACT FIFO and the boundary costs no ACT latency
                    if late_q and not (h == 0 and g == 0):
                        late_q.pop(0)()
                    if g == j:
                        nc.vector.tensor_mul(pt, pt, mask_t[:, j])
                    for i in range(4):
                        kb = 4 * g + i
                        nc.tensor.matmul(yh, vp_t[h][:, kb],
                                         pt[:, QC * i:QC * (i + 1)],
                                         start=(g == 0 and i == 0),
                                         stop=(g == j and i == 3))
                    sg_cur = sg_next
                nc.vector.tensor_copy(den3[:, QC * h:QC * (h + 1)], yh[64:65])
            while late_q:
                late_q.pop(0)()

            # stage unnormalized y' to SBUF: frees yacc for the next slot's
            # PV immediately, and lets the whole normalize chain defer
            ystage = ysp.tile([64, NH * QC], f32, tag="ys", name="ys")
            nc.vector.tensor_copy(ystage, yacc[0:64, :])

            while qh_q:     # next slot's Q must exist before its S-groups
                qh_q.pop(0)()

            # normalized y^T packed into two head-pair tiles for the
            # K=128 paired projection ([y0;y1] and [y2;y2] w/ halved W)
            ytpair = [ytp.tile([P, QC], f16, tag=f"ytp{p}", name=f"ytp{p}")
                      for p in range(2)]

            def norm_chunk(den3=den3, ystage=ystage, ytpair=ytpair):
                # 1/den: ln then exp(-x) on ACT (one shared table set)
                l3 = rbp.tile([1, NH * QC], f32, tag="l3", name="l3")
                nc.scalar.activation(l3, den3, LN)
                r3 = rbp.tile([1, NH * QC], f32r, tag="r3", name="r3")
                nc.scalar.activation(r3, l3, EXP, scale=-1.0)
                for h in range(NH):
                    bc = psm.tile([64, QC], f32, tag="psm", name="bc")
                    nc.tensor.matmul(bc, ones64, r3[:, QC * h:QC * (h + 1)],
                                     start=True, stop=True)
                    rb = rbp.tile([64, QC], f32, tag=f"rb{h}", name=f"rb{h}")
                    nc.vector.tensor_copy(rb, bc)
                    dsts = ([ytpair[h // 2][64 * (h % 2):64 * (h % 2) + 64]]
                            if h < 2 else
                            [ytpair[1][0:64], ytpair[1][64:P]])
                    for dst in dsts:
                        nc.vector.tensor_mul(
                            dst, ystage[:, QC * h:QC * (h + 1)], rb)

            ost = ostp.tile([P, 2, C], f32, tag="ost", name="ost")
            projwork = []
            for qb in range(2):
                for ci, (n0, nw) in enumerate([(0, 512), (512, 256)]):
                    def run(qb=qb, n0=n0, nw=nw, ci=ci, ost=ost,
                            ytpair=ytpair, j=j):
                        pp = psm.tile([P, 512], f32, tag="psm",
                                      name="pp")[:, :nw]
                        for pr in range(2):
                            nc.tensor.matmul(
                                pp, ytpair[pr][:, P * qb:P * (qb + 1)],
                                wpj_t[:, pr, n0:n0 + nw],
                                start=(pr == 0), stop=(pr == 1))
                        nc.vector.tensor_copy(ost[:, qb, n0:n0 + nw], pp)
                        if ci == 1:
                            nc.sync.dma_start(
                                out[QC * j + P * qb:QC * j + P * (qb + 1), :],
                                ost[:, qb])
                    projwork.append(run)
            if j < nslot - 1:
                # normalize+projection drain inside the next slot's
                # attention loop, after its exps are already in the ACT FIFO
                late_q.append(norm_chunk)
                late_q.extend(projwork)
            else:
                norm_chunk()
                for w in projwork:
                    w()

        # interleave: slots 2q,2q+1 only need kv from quarters <= q, so
        # quarter q+1's chunks drain between their attention groups and the
        # QKV production rides inside the PE's exp-wait gaps
        for j in range(nslot):
            if j % 2 == 0:
                if j >= 2:
                    while work_q:   # previous quarter must be complete
                        work_q.pop(0)()
                if j // 2 + 1 < 4:
                    work_q.extend(make_quarter_work(j // 2 + 1))
            do_slot(j)
        while work_q:
            work_q.pop(0)()

    split_multi_waits(nc)
    return nc


def make_in_maps(x, W_qkv, b_qkv, W_proj, Tloc=T):
    """Shard the full inputs into the 8 per-core input maps."""
    nslot = Tloc // (2 * QC)
    tq = nslot * QC
    xT = np.ascontiguousarray(x.reshape(Tloc, C).T).astype(np.float32)
    xT16 = xT.astype(np.float16)
    # [C, T] -> [128, 6, T] (c-block-major partitions)
    xt_l = np.ascontiguousarray(xT16.reshape(6, P, Tloc).transpose(1, 0, 2))

    kk = np.arange(P)
    qq = np.arange(QC)
    in_maps = []
    for core in range(8):
        hg, s = core // 2, core % 2
        heads = [3 * hg + i for i in range(NH)]
        wk = [W_qkv[:, C + 64 * h:C + 64 * h + 64] for h in heads]
        wv = [W_qkv[:, 2 * C + 64 * h:2 * C + 64 * h + 64] for h in heads]
        wkv_c = np.concatenate(wk + wv, axis=1)          # [768, 384]
        wq_c = np.concatenate(
            [np.tile(W_qkv[:, 64 * h:64 * h + 64], (1, 2)) for h in heads],
            axis=1)                                      # [768, 384]
        # head-paired projection: [W_h0; W_h1] and [W_h2/2; W_h2/2]
        wpj_c = np.stack([
            np.concatenate([W_proj[64 * heads[0]:64 * heads[0] + 64, :],
                            W_proj[64 * heads[1]:64 * heads[1] + 64, :]], axis=0),
            np.tile(0.5 * W_proj[64 * heads[2]:64 * heads[2] + 64, :], (2, 1)),
        ], axis=1)                                       # [128, 2, 768]
        wkv_16 = np.ascontiguousarray(
            wkv_c.reshape(6, P, 6 * D).transpose(1, 0, 2)).astype(np.float16)
        wq_16 = np.ascontiguousarray(
            wq_c.reshape(6, P, 2 * NH * D).transpose(1, 0, 2)).astype(np.float16)
        wpj_16 = np.ascontiguousarray(wpj_c).astype(np.float16)

        bk = [b_qkv[C + 64 * h:C + 64 * h + 64] for h in heads]
        bv = [b_qkv[2 * C + 64 * h:2 * C + 64 * h + 64] for h in heads]
        bkv_c = np.concatenate(bk + bv)          # [384]
        bias_c = np.zeros((P, 6), np.float32)
        bias_c[:, 0:3] = bkv_c.reshape(3, P).T
        for hi_, h in enumerate(heads):
            bias_c[0:64, 3 + hi_] = b_qkv[64 * h:64 * h + 64]
            bias_c[64:P, 3 + hi_] = b_qkv[64 * h:64 * h + 64]

        qcols = np.concatenate(
            [np.arange(QC * (2 * j + s), QC * (2 * j + s) + QC) for j in range(nslot)])
        xtq_16 = np.ascontiguousarray(
            xT16[:, qcols].reshape(6, P, tq).transpose(1, 0, 2))

        mask_c = np.zeros((nslot, P, 1024), np.float32)
        for j in range(nslot):
            q0 = QC * (2 * j + s)
            for i in range(4):
                k0 = P * (4 * j + i)
                mask_c[j, :, QC * i:QC * (i + 1)] = (
                    (k0 + kk[:, None]) <= (q0 + qq[None, :]))
        mask_l = np.ascontiguousarray(mask_c.transpose(1, 0, 2)).astype(np.float16)

        in_maps.append({
            "xt16": xt_l, "xtq16": xtq_16,
            "wkv16": wkv_16, "wq16": wq_16, "wpj16": wpj_16,
            "bias": bias_c, "mask": mask_l,
        })
    return in_maps


def unshard(results, b_proj, Tloc=T):
    nslot = Tloc // (2 * QC)
    out = np.zeros((Tloc, C), np.float64)
    for core in range(8):
        s = core % 2
        r = results[core]["out"].astype(np.float64)
        for j in range(nslot):
            g0 = QC * (2 * j + s)
            out[g0:g0 + QC] += r[QC * j:QC * (j + 1)]
    out += b_proj.astype(np.float64)
    return out.astype(np.float32).reshape(1, Tloc, C)


_last_result = {}


def kernel(x, mask, W_qkv, b_qkv, W_proj, b_proj):
    from concourse.bass_utils import run_bass_kernel_spmd
    x = np.asarray(x, np.float32)
    W_qkv = np.asarray(W_qkv, np.float32)
    b_qkv = np.asarray(b_qkv, np.float32)
    W_proj = np.asarray(W_proj, np.float32)
    b_proj = np.asarray(b_proj, np.float32)

    if "nc" not in _nc_cache:
        _nc_cache["nc"] = build_nc(T)
    nc = _nc_cache["nc"]
    in_maps = make_in_maps(x, W_qkv, b_qkv, W_proj, T)
    import os
    kwargs = {}
    if os.environ.get("BASS_KERNEL_TRACE"):
        kwargs = dict(trace=True, trace_cores=list(range(8)))
    res = run_bass_kernel_spmd(nc, in_maps, core_ids=list(range(8)), **kwargs)
    _last_result["res"] = res
    return unshard([r for r in res.results], b_proj, T)


# revision 57
# speedup vs baseline: 1.0188x; 1.0188x over previous
"""Causal self-attention (B=1, T=4096, C=768, H=12, D=64) on 8 TRN2 NeuronCores.

Sharding: 8 cores = 4 head-groups (3 heads each) x 2 sequence-groups.
Core c: heads [3*hg, 3*hg+2] where hg=c//2; handles q-chunks of 256 rows,
global chunk g = 2*j + s (s=c%2, j=0..7) -- interleaving balances the causal
triangle so every core runs an identical instruction stream (SPMD), with the
boundary masks supplied as per-core data.

All matmuls fp16 single-pass (fp32 PSUM accumulation).  Flash-style
attention in S^T = K@Q^T orientation; the softmax denominator arrives free
as the ones-column row of (V' P^T).  Causality = block skipping + 0/1
boundary masks.  Every matmul keeps 128-partition operands (diag-packed
K^T, doubled Q, head-paired projection): the PE activity monitor throttles
the clock to 1.2 GHz when it sees half-idle arrays, so full-width operands
are a 2x clock win, not a FLOP nicety.

Perf structure: all inputs DMA'd up front in single transfers; phase-1 V
transposes software-pipelined one matmul-group behind; per-slot tail does
den -> ACT ln -> ACT exp(-x) (reciprocal off the Vector engine) hidden
behind next-slot Q matmuls, then the head-paired projection.
Host sums the 4 head-group partial projections.
"""
import numpy as np

T, C, H, D = 4096, 768, 12, 64
NH = 3          # heads per core
QC = 256        # q rows per slot
P = 128

_nc_cache = {}


def split_multi_waits(nc):
    """Walrus here accepts only one sync wait per instruction: hoist extras
    onto standalone InstEventSemaphore instructions on the same engine."""
    import concourse.mybir as mybir
    n_split = 0
    for f in nc.m.functions:
        for bb in f.blocks:
            new_insts = []
            for inst in bb.instructions:
                si = inst.sync_info
                if si is not None and len(si.on_wait) > 1:
                    for w in si.on_wait[:-1]:
                        nop = mybir.InstEventSemaphore(
                            name=nc.get_next_instruction_name(), ins=[], outs=[])
                        nop.engine = inst.engine
                        nop.sync_info = mybir.SyncInfo(on_wait=[w], on_update=[])
                        nc.register_instruction(nop)
                        new_insts.append(nop)
                        n_split += 1
                    si.on_wait = si.on_wait[-1:]
                new_insts.append(inst)
            bb.instructions[:] = new_insts
    return n_split


def build_nc(Tloc=T):
    import concourse.bass as bass
    import concourse.mybir as mybir
    import concourse.tile as tile
    from concourse.masks import make_identity
    from contextlib import ExitStack

    f32r = mybir.dt.float32r
    f32 = mybir.dt.float32
    f16 = mybir.dt.float16
    EXP = mybir.ActivationFunctionType.Exp
    LN = mybir.ActivationFunctionType.Ln
    CPY = mybir.ActivationFunctionType.Copy
    ADD = mybir.AluOpType.add

    nslot = Tloc // (2 * QC)     # q-chunks per core
    nkb = Tloc // P              # k 128-blocks
    tq = nslot * QC              # q rows per core

    nc = bass.Bass(trn_type="TRN2")
    xt16 = nc.dram_tensor("xt16", [P, 6, Tloc], f16, kind="ExternalInput")
    xtq16 = nc.dram_tensor("xtq16", [P, 6, tq], f16, kind="ExternalInput")
    wkv16 = nc.dram_tensor("wkv16", [P, 6, 6 * D], f16, kind="ExternalInput")
    wq16 = nc.dram_tensor("wq16", [P, 6, 2 * NH * D], f16, kind="ExternalInput")
    wpj16 = nc.dram_tensor("wpj16", [P, 2, C], f16, kind="ExternalInput")
    bias = nc.dram_tensor("bias", [P, 6], f32, kind="ExternalInput")
    mask = nc.dram_tensor("mask", [P, nslot, 1024], f16, kind="ExternalInput")
    out = nc.dram_tensor("out", [tq, C], f32, kind="ExternalOutput")

    # m-chunk -> (kind, head) for the packed [k0 k1 k2 v0 v1 v2] KV weights
    kv_map = [[("K", 0), ("K", 1)], [("K", 2), ("V", 0)], [("V", 1), ("V", 2)]]

    with tile.TileContext(nc) as tc, ExitStack() as ctx:
        singles = ctx.enter_context(tc.tile_pool(name="singles", bufs=1))
        ysp = ctx.enter_context(tc.tile_pool(name="ys", bufs=2))
        xthp = ctx.enter_context(tc.tile_pool(name="xth", bufs=2))
        vstp = ctx.enter_context(tc.tile_pool(name="vst", bufs=2))
        qtp = ctx.enter_context(tc.tile_pool(name="qt", bufs=2))
        ptp = ctx.enter_context(tc.tile_pool(name="pt", bufs=3))
        rbp = ctx.enter_context(tc.tile_pool(name="rb", bufs=2))
        ytp = ctx.enter_context(tc.tile_pool(name="yt", bufs=2))
        ostp = ctx.enter_context(tc.tile_pool(name="ost", bufs=2))
        psg = ctx.enter_context(tc.tile_pool(name="psg", bufs=2, space="PSUM"))
        psy = ctx.enter_context(tc.tile_pool(name="psy", bufs=1, space="PSUM"))
        psm = ctx.enter_context(tc.tile_pool(name="psm", bufs=2, space="PSUM"))

        # --- primary weights first so phase 1 can start the moment x lands
        wkv_t = singles.tile([P, 6, 6 * D], f16)
        nc.sync.dma_start(wkv_t, wkv16[:, :, :])
        b_t = singles.tile([P, 6], f32)
        nc.sync.dma_start(b_t, bias[:, :])

        ident16 = singles.tile([P, P], f16)
        make_identity(nc, ident16)
        ones_f = singles.tile([1, 64], f32)
        nc.vector.memset(ones_f, 1.0)
        ones64 = singles.tile([1, 64], f32r)
        nc.vector.tensor_copy(ones64, ones_f)
        onesk_f = singles.tile([P, 32], f16)
        nc.vector.memset(onesk_f, 1.0)

        kt_t = [singles.tile([P, Tloc], f16, tag=f"kt{h}", name=f"kt{h}")
                for h in range(NH)]
        # bottom partition half starts zeroed; odd 64-col halves get K^T rows
        # DMA'd down, then the top copies of those columns are zeroed, giving
        # diag(K^T even-half, K^T odd-half) per 128-col block -> K=128 matmuls
        for h in range(NH):
            nc.vector.memset(kt_t[h][64:P, :], 0.0)
        vp_t = [singles.tile([P, nkb, 65], f16, tag=f"vp{h}", name=f"vp{h}")
                for h in range(NH)]
        for h in range(NH):
            nc.vector.tensor_copy(vp_t[h][:, :, 64], onesk_f[:, :nkb])

        # ---- Phase 1: K^T, V-transposed from x^T, in 4 column-quarters ----
        QT4 = Tloc // 4
        pending = []            # deferred V transposes: (m, g0, vst)

        def flush_pending():
            for (m, g0, vst) in pending:
                for i in range(4):
                    tp = psm.tile([P, P], f16, tag="psm", name="tp")
                    nc.tensor.transpose(tp, vst[:, P * i:P * (i + 1)], ident16)
                    blk = g0 // P + i
                    for sub, (kind, h) in enumerate(kv_map[m]):
                        if kind == "V":
                            nc.scalar.activation(
                                vp_t[h][:, blk, 0:64],
                                tp[:, 64 * sub:64 * sub + 64], CPY)
            pending.clear()

        def mm_chunk(quarter, m, n0, xh_box):
            xh = xh_box[0]
            ps = psg.tile([P, 1024], f32, tag="sg", name="ps")[:, :512]
            for c in range(6):
                nc.tensor.matmul(ps, wkv_t[:, c, P * m:P * m + P],
                                 xh[:, c, n0:n0 + 512],
                                 start=(c == 0), stop=(c == 5))
            flush_pending()
            g0 = QT4 * quarter + n0
            has_v = any(kind == "V" for kind, _ in kv_map[m])
            if has_v:
                # full-block bias-add (per-partition bias covers both
                # 64-row subs); K rows are also staged but unused
                vst = vstp.tile([P, 512], f16, tag="vst", name="vst")
                nc.vector.tensor_scalar(vst, ps, b_t[:, m:m + 1], None, ADD)
                pending.append((m, g0, vst))
            for sub, (kind, h) in enumerate(kv_map[m]):
                if kind == "K":
                    rows = slice(64 * sub, 64 * sub + 64)
                    nc.vector.tensor_scalar(
                        kt_t[h][0:64, g0:g0 + 512], ps[rows],
                        b_t[rows, m:m + 1], None, ADD)

        def kt_fixup(b0, nb=8):
            # diag-pack fixup for k-blocks [b0, b0+nb), so the slots that
            # only need kv up to here can start immediately
            flush_pending()
            for h in range(NH):
                ot = kt_t[h][0:64].rearrange(
                    "p (b t) -> p b t", t=P)[:, b0:b0 + nb, 64:P]
                ob = kt_t[h][64:P].rearrange(
                    "p (b t) -> p b t", t=P)[:, b0:b0 + nb, 64:P]
                nc.sync.dma_start(ob, ot)
                nc.vector.memset(ot, 0.0)

        def make_quarter_work(quarter):
            """Quarter q as a list of small closures, drained one per
            attention group of the two preceding slots."""
            xh_box = [None]

            def dma():
                xh = xthp.tile([P, 6, QT4], f16, tag="xh", name="xh")
                nc.sync.dma_start(
                    xh, xt16[:, :, QT4 * quarter:QT4 * (quarter + 1)])
                xh_box[0] = xh
            items = [dma]
            for m in range(3):
                for n0 in (0, 512):
                    items.append(
                        lambda m=m, n0=n0: mm_chunk(quarter, m, n0, xh_box))
            items.append(lambda: kt_fixup(8 * quarter))
            return items

        # quarter 0 runs up front (slots 0-1 need it immediately)
        xh0_box = [None]
        xh0 = xthp.tile([P, 6, QT4], f16, tag="xh", name="xh")
        nc.sync.dma_start(xh0, xt16[:, :, 0:QT4])
        xh0_box[0] = xh0
        wq_t = singles.tile([P, 6, 2 * NH * D], f16)
        nc.sync.dma_start(wq_t, wq16[:, :, :])
        xtq_t = singles.tile([P, 6, tq], f16)
        nc.sync.dma_start(xtq_t, xtq16[:, :, :])
        mask_t = singles.tile([P, nslot, 1024], f16)
        nc.sync.dma_start(mask_t, mask[:, :, :])
        wpj_t = singles.tile([P, 2, C], f16)
        nc.sync.dma_start(wpj_t, wpj16[:, :, :])
        # slot 0 only needs k-blocks 0-3: run just the first half of
        # quarter 0 up front, defer the second half into slot 0's drains
        for m in range(3):
            mm_chunk(0, m, 0, xh0_box)
        kt_fixup(0, 4)

        # deferred phase-1 chunks (quarter-0 second half now; quarters 1-3
        # and deferred projections join later)
        work_q = [lambda m=m: mm_chunk(0, m, 512, xh0_box) for m in range(3)]
        work_q.append(lambda: kt_fixup(4, 4))
        late_q = []     # previous slot's normalize/projection chunks
        qh_q = []       # next slot's Q-head chunks

        # ---- Phase 2: per q-slot: attention, then (next Q | recip | proj) ----
        qt_cur = [None] * NH

        def q_head(j, h):
            psq = psm.tile([P, QC], f32, tag="psm", name="psq")
            for c in range(6):
                nc.tensor.matmul(psq, wq_t[:, c, P * h:P * (h + 1)],
                                 xtq_t[:, c, QC * j:QC * (j + 1)],
                                 start=(c == 0), stop=(c == 5))
            qh = qtp.tile([P, QC], f16, tag=f"qt{h}", name=f"qt{h}")
            nc.vector.tensor_scalar(qh, psq, b_t[:, 3 + h:4 + h], None, ADD)
            qt_cur[h] = qh

        def q_block(j):
            for h in range(NH):
                q_head(j, h)

        q_block(0)

        def do_slot(j):
            yacc = psy.tile([65, NH * QC], f32, tag="yacc", name="yacc")
            den3 = rbp.tile([1, NH * QC], f32, tag="den", name="den")
            my_qt = list(qt_cur)

            for h in range(NH):
                yh = yacc[:, QC * h:QC * (h + 1)]

                def s_group(g):
                    sg = psg.tile([P, 1024], f32, tag="sg", name="sg")
                    for i in range(4):
                        kb = 4 * g + i
                        nc.tensor.matmul(sg[:, QC * i:QC * (i + 1)],
                                         kt_t[h][:, P * kb:P * (kb + 1)],
                                         my_qt[h], start=True, stop=True)
                    return sg

                # software pipeline: issue S(g+1) before PV(g) so the PE has
                # work while ACT runs exp(g)
                sg_cur = s_group(0)
                for g in range(j + 1):
                    # drain BEFORE issuing sg_next: a pending kt fixup must
                    # be emitted before any S-group that reads its blocks
                    if work_q:
                        work_q.pop(0)()
                    sg_next = s_group(g + 1) if g < j else None
                    pt = ptp.tile([P, 1024], f16, tag="pt", name="pt")
                    nc.scalar.activation(pt, sg_cur, EXP, scale=0.125)
                    # previous slot's normalize/project chunks drain AFTER
                    # this exp is emitted, so their Ln/Exp land behind it in
                    # the ACT FIFO and the boundary costs no ACT latency
                    if late_q and not (h == 0 and g == 0):
                        late_q.pop(0)()
                    if g == j:
                        nc.vector.tensor_mul(pt, pt, mask_t[:, j])
                    for i in range(4):
                        kb = 4 * g + i
                        nc.tensor.matmul(yh, vp_t[h][:, kb],
                                         pt[:, QC * i:QC * (i + 1)],
                                         start=(g == 0 and i == 0),
                                         stop=(g == j and i == 3))
                    sg_cur = sg_next
                nc.vector.tensor_copy(den3[:, QC * h:QC * (h + 1)], yh[64:65])
            while late_q:
                late_q.pop(0)()

            # stage unnormalized y' to SBUF: frees yacc for the next slot's
            # PV immediately, and lets the whole normalize chain defer
            ystage = ysp.tile([64, NH * QC], f32, tag="ys", name="ys")
            nc.vector.tensor_copy(ystage, yacc[0:64, :])

            if j + 1 < nslot:
                q_block(j + 1)

            # normalized y^T packed into two head-pair tiles for the
            # K=128 paired projection ([y0;y1] and [y2;y2] w/ halved W)
            ytpair = [ytp.tile([P, QC], f16, tag=f"ytp{p}", name=f"ytp{p}")
                      for p in range(2)]

            def norm_chunk(den3=den3, ystage=ystage, ytpair=ytpair):
                # 1/den: ln then exp(-x) on ACT (one shared table set)
                l3 = rbp.tile([1, NH * QC], f32, tag="l3", name="l3")
                nc.scalar.activation(l3, den3, LN)
                r3 = rbp.tile([1, NH * QC], f32r, tag="r3", name="r3")
                nc.scalar.activation(r3, l3, EXP, scale=-1.0)
                for h in range(NH):
                    bc = psm.tile([64, QC], f32, tag="psm", name="bc")
                    nc.tensor.matmul(bc, ones64, r3[:, QC * h:QC * (h + 1)],
                                     start=True, stop=True)
                    rb = rbp.tile([64, QC], f32, tag=f"rb{h}", name=f"rb{h}")
                    nc.vector.tensor_copy(rb, bc)
                    dsts = ([ytpair[h // 2][64 * (h % 2):64 * (h % 2) + 64]]
                            if h < 2 else
                            [ytpair[1][0:64], ytpair[1][64:P]])
                    for dst in dsts:
                        nc.vector.tensor_mul(
                            dst, ystage[:, QC * h:QC * (h + 1)], rb)

            ost = ostp.tile([P, 2, C], f32, tag="ost", name="ost")
            projwork = []
            for qb in range(2):
                for ci, (n0, nw) in enumerate([(0, 512), (512, 256)]):
                    def run(qb=qb, n0=n0, nw=nw, ci=ci, ost=ost,
                            ytpair=ytpair, j=j):
                        pp = psm.tile([P, 512], f32, tag="psm",
                                      name="pp")[:, :nw]
                        for pr in range(2):
                            nc.tensor.matmul(
                                pp, ytpair[pr][:, P * qb:P * (qb + 1)],
                                wpj_t[:, pr, n0:n0 + nw],
                                start=(pr == 0), stop=(pr == 1))
                        nc.vector.tensor_copy(ost[:, qb, n0:n0 + nw], pp)
                        if ci == 1:
                            nc.sync.dma_start(
                                out[QC * j + P * qb:QC * j + P * (qb + 1), :],
                                ost[:, qb])
                    projwork.append(run)
            if j < nslot - 1:
                # normalize+projection drain inside the next slot's
                # attention loop, after its exps are already in the ACT FIFO
                late_q.append(norm_chunk)
                late_q.extend(projwork)
            else:
                norm_chunk()
                for w in projwork:
                    w()

        # interleave: slots 2q,2q+1 only need kv from quarters <= q, so
        # quarter q+1's chunks drain between their attention groups and the
        # QKV production rides inside the PE's exp-wait gaps
        for j in range(nslot):
            if j % 2 == 0:
                if j >= 2:
                    while work_q:   # previous quarter must be complete
                        work_q.pop(0)()
                if j // 2 + 1 < 4:
                    work_q.extend(make_quarter_work(j // 2 + 1))
            do_slot(j)
        while work_q:
            work_q.pop(0)()

    split_multi_waits(nc)
    return nc


def make_in_maps(x, W_qkv, b_qkv, W_proj, Tloc=T):
    """Shard the full inputs into the 8 per-core input maps."""
    nslot = Tloc // (2 * QC)
    tq = nslot * QC
    xT = np.ascontiguousarray(x.reshape(Tloc, C).T).astype(np.float32)
    xT16 = xT.astype(np.float16)
    # [C, T] -> [128, 6, T] (c-block-major partitions)
    xt_l = np.ascontiguousarray(xT16.reshape(6, P, Tloc).transpose(1, 0, 2))

    kk = np.arange(P)
    qq = np.arange(QC)
    in_maps = []
    for core in range(8):
        hg, s = core // 2, core % 2
        heads = [3 * hg + i for i in range(NH)]
        wk = [W_qkv[:, C + 64 * h:C + 64 * h + 64] for h in heads]
        wv = [W_qkv[:, 2 * C + 64 * h:2 * C + 64 * h + 64] for h in heads]
        wkv_c = np.concatenate(wk + wv, axis=1)          # [768, 384]
        wq_c = np.concatenate(
            [np.tile(W_qkv[:, 64 * h:64 * h + 64], (1, 2)) for h in heads],
            axis=1)                                      # [768, 384]
        # head-paired projection: [W_h0; W_h1] and [W_h2/2; W_h2/2]
        wpj_c = np.stack([
            np.concatenate([W_proj[64 * heads[0]:64 * heads[0] + 64, :],
                            W_proj[64 * heads[1]:64 * heads[1] + 64, :]], axis=0),
            np.tile(0.5 * W_proj[64 * heads[2]:64 * heads[2] + 64, :], (2, 1)),
        ], axis=1)                                       # [128, 2, 768]
        wkv_16 = np.ascontiguousarray(
            wkv_c.reshape(6, P, 6 * D).transpose(1, 0, 2)).astype(np.float16)
        wq_16 = np.ascontiguousarray(
            wq_c.reshape(6, P, 2 * NH * D).transpose(1, 0, 2)).astype(np.float16)
        wpj_16 = np.ascontiguousarray(wpj_c).astype(np.float16)

        bk = [b_qkv[C + 64 * h:C + 64 * h + 64] for h in heads]
        bv = [b_qkv[2 * C + 64 * h:2 * C + 64 * h + 64] for h in heads]
        bkv_c = np.concatenate(bk + bv)          # [384]
        bias_c = np.zeros((P, 6), np.float32)
        bias_c[:, 0:3] = bkv_c.reshape(3, P).T
        for hi_, h in enumerate(heads):
            bias_c[0:64, 3 + hi_] = b_qkv[64 * h:64 * h + 64]
            bias_c[64:P, 3 + hi_] = b_qkv[64 * h:64 * h + 64]

        qcols = np.concatenate(
            [np.arange(QC * (2 * j + s), QC * (2 * j + s) + QC) for j in range(nslot)])
        xtq_16 = np.ascontiguousarray(
            xT16[:, qcols].reshape(6, P, tq).transpose(1, 0, 2))

        mask_c = np.zeros((nslot, P, 1024), np.float32)
        for j in range(nslot):
            q0 = QC * (2 * j + s)
            for i in range(4):
                k0 = P * (4 * j + i)
                mask_c[j, :, QC * i:QC * (i + 1)] = (
                    (k0 + kk[:, None]) <= (q0 + qq[None, :]))
        mask_l = np.ascontiguousarray(mask_c.transpose(1, 0, 2)).astype(np.float16)

        in_maps.append({
            "xt16": xt_l, "xtq16": xtq_16,
            "wkv16": wkv_16, "wq16": wq_16, "wpj16": wpj_16,
            "bias": bias_c, "mask": mask_l,
        })
    return in_maps


def unshard(results, b_proj, Tloc=T):
    nslot = Tloc // (2 * QC)
    out = np.zeros((Tloc, C), np.float64)
    for core in range(8):
        s = core % 2
        r = results[core]["out"].astype(np.float64)
        for j in range(nslot):
            g0 = QC * (2 * j + s)
            out[g0:g0 + QC] += r[QC * j:QC * (j + 1)]
    out += b_proj.astype(np.float64)
    return out.astype(np.float32).reshape(1, Tloc, C)


_last_result = {}


def kernel(x, mask, W_qkv, b_qkv, W_proj, b_proj):
    from concourse.bass_utils import run_bass_kernel_spmd
    x = np.asarray(x, np.float32)
    W_qkv = np.asarray(W_qkv, np.float32)
    b_qkv = np.asarray(b_qkv, np.float32)
    W_proj = np.asarray(W_proj, np.float32)
    b_proj = np.asarray(b_proj, np.float32)

    if "nc" not in _nc_cache:
        _nc_cache["nc"] = build_nc(T)
    nc = _nc_cache["nc"]
    in_maps = make_in_maps(x, W_qkv, b_qkv, W_proj, T)
    import os
    kwargs = {}
    if os.environ.get("BASS_KERNEL_TRACE"):
        kwargs = dict(trace=True, trace_cores=list(range(8)))
    res = run_bass_kernel_spmd(nc, in_maps, core_ids=list(range(8)), **kwargs)
    _last_result["res"] = res
    return unshard([r for r in res.results], b_proj, T)


# revision 62
# speedup vs baseline: 1.0195x; 1.0007x over previous
"""Causal self-attention (B=1, T=4096, C=768, H=12, D=64) on 8 TRN2 NeuronCores.

Sharding: 8 cores = 4 head-groups (3 heads each) x 2 sequence-groups.
Core c: heads [3*hg, 3*hg+2] where hg=c//2; handles q-chunks of 256 rows,
global chunk g = 2*j + s (s=c%2, j=0..7) -- interleaving balances the causal
triangle so every core runs an identical instruction stream (SPMD), with the
boundary masks supplied as per-core data.

All matmuls fp16 single-pass (fp32 PSUM accumulation).  Flash-style
attention in S^T = K@Q^T orientation; the softmax denominator arrives free
as the ones-column row of (V' P^T).  Causality = block skipping + 0/1
boundary masks.  Every matmul keeps 128-partition operands (diag-packed
K^T, doubled Q, head-paired projection): the PE activity monitor throttles
the clock to 1.2 GHz when it sees half-idle arrays, so full-width operands
are a 2x clock win, not a FLOP nicety.

Perf structure: all inputs DMA'd up front in single transfers; phase-1 V
transposes software-pipelined one matmul-group behind; per-slot tail does
den -> ACT ln -> ACT exp(-x) (reciprocal off the Vector engine) hidden
behind next-slot Q matmuls, then the head-paired projection.
Host sums the 4 head-group partial projections.
"""
import numpy as np

T, C, H, D = 4096, 768, 12, 64
NH = 3          # heads per core
QC = 256        # q rows per slot
P = 128

_nc_cache = {}


def split_multi_waits(nc):
    """Walrus here accepts only one sync wait per instruction: hoist extras
    onto standalone InstEventSemaphore instructions on the same engine."""
    import concourse.mybir as mybir
    n_split = 0
    for f in nc.m.functions:
        for bb in f.blocks:
            new_insts = []
            for inst in bb.instructions:
                si = inst.sync_info
                if si is not None and len(si.on_wait) > 1:
                    for w in si.on_wait[:-1]:
                        nop = mybir.InstEventSemaphore(
                            name=nc.get_next_instruction_name(), ins=[], outs=[])
                        nop.engine = inst.engine
                        nop.sync_info = mybir.SyncInfo(on_wait=[w], on_update=[])
                        nc.register_instruction(nop)
                        new_insts.append(nop)
                        n_split += 1
                    si.on_wait = si.on_wait[-1:]
                new_insts.append(inst)
            bb.instructions[:] = new_insts
    return n_split


def build_nc(Tloc=T):
    import concourse.bass as bass
    import concourse.mybir as mybir
    import concourse.tile as tile
    from concourse.masks import make_identity
    from contextlib import ExitStack

    f32r = mybir.dt.float32r
    f32 = mybir.dt.float32
    f16 = mybir.dt.float16
    EXP = mybir.ActivationFunctionType.Exp
    LN = mybir.ActivationFunctionType.Ln
    CPY = mybir.ActivationFunctionType.Copy
    ADD = mybir.AluOpType.add

    nslot = Tloc // (2 * QC)     # q-chunks per core
    nkb = Tloc // P              # k 128-blocks
    tq = nslot * QC              # q rows per core

    nc = bass.Bass(trn_type="TRN2")
    xt16 = nc.dram_tensor("xt16", [P, 6, Tloc], f16, kind="ExternalInput")
    xtq16 = nc.dram_tensor("xtq16", [P, 6, tq], f16, kind="ExternalInput")
    wkv16 = nc.dram_tensor("wkv16", [P, 6, 6 * D], f16, kind="ExternalInput")
    wq16 = nc.dram_tensor("wq16", [P, 6, 2 * NH * D], f16, kind="ExternalInput")
    wpj16 = nc.dram_tensor("wpj16", [P, 2, C], f16, kind="ExternalInput")
    bias = nc.dram_tensor("bias", [P, 6], f32, kind="ExternalInput")
    mask = nc.dram_tensor("mask", [P, nslot, 1024], f16, kind="ExternalInput")
    out = nc.dram_tensor("out", [tq, C], f32, kind="ExternalOutput")

    # m-chunk -> (kind, head) for the packed [k0 k1 k2 v0 v1 v2] KV weights
    kv_map = [[("K", 0), ("K", 1)], [("K", 2), ("V", 0)], [("V", 1), ("V", 2)]]

    with tile.TileContext(nc) as tc, ExitStack() as ctx:
        singles = ctx.enter_context(tc.tile_pool(name="singles", bufs=1))
        ysp = ctx.enter_context(tc.tile_pool(name="ys", bufs=2))
        xthp = ctx.enter_context(tc.tile_pool(name="xth", bufs=2))
        vstp = ctx.enter_context(tc.tile_pool(name="vst", bufs=2))
        qtp = ctx.enter_context(tc.tile_pool(name="qt", bufs=2))
        ptp = ctx.enter_context(tc.tile_pool(name="pt", bufs=3))
        rbp = ctx.enter_context(tc.tile_pool(name="rb", bufs=2))
        ytp = ctx.enter_context(tc.tile_pool(name="yt", bufs=2))
        ostp = ctx.enter_context(tc.tile_pool(name="ost", bufs=2))
        psg = ctx.enter_context(tc.tile_pool(name="psg", bufs=2, space="PSUM"))
        psy = ctx.enter_context(tc.tile_pool(name="psy", bufs=1, space="PSUM"))
        psm = ctx.enter_context(tc.tile_pool(name="psm", bufs=2, space="PSUM"))

        # --- primary weights first so phase 1 can start the moment x lands
        wkv_t = singles.tile([P, 6, 6 * D], f16)
        nc.sync.dma_start(wkv_t, wkv16[:, :, :])
        b_t = singles.tile([P, 6], f32)
        nc.sync.dma_start(b_t, bias[:, :])

        ident16 = singles.tile([P, P], f16)
        make_identity(nc, ident16)
        ones_f = singles.tile([1, 64], f32)
        nc.vector.memset(ones_f, 1.0)
        ones64 = singles.tile([1, 64], f32r)
        nc.vector.tensor_copy(ones64, ones_f)
        onesk_f = singles.tile([P, 32], f16)
        nc.vector.memset(onesk_f, 1.0)

        kt_t = [singles.tile([P, Tloc], f16, tag=f"kt{h}", name=f"kt{h}")
                for h in range(NH)]
        # bottom partition half starts zeroed; odd 64-col halves get K^T rows
        # DMA'd down, then the top copies of those columns are zeroed, giving
        # diag(K^T even-half, K^T odd-half) per 128-col block -> K=128 matmuls
        for h in range(NH):
            nc.vector.memset(kt_t[h][64:P, :], 0.0)
        vp_t = [singles.tile([P, nkb, 65], f16, tag=f"vp{h}", name=f"vp{h}")
                for h in range(NH)]
        for h in range(NH):
            nc.vector.tensor_copy(vp_t[h][:, :, 64], onesk_f[:, :nkb])

        # ---- Phase 1: K^T, V-transposed from x^T, in 4 column-quarters ----
        QT4 = Tloc // 4
        pending = []            # deferred V transposes: (m, g0, vst)

        def flush_pending():
            for (m, g0, vst) in pending:
                for i in range(4):
                    tp = psm.tile([P, P], f16, tag="psm", name="tp")
                    nc.tensor.transpose(tp, vst[:, P * i:P * (i + 1)], ident16)
                    blk = g0 // P + i
                    for sub, (kind, h) in enumerate(kv_map[m]):
                        if kind == "V":
                            nc.scalar.activation(
                                vp_t[h][:, blk, 0:64],
                                tp[:, 64 * sub:64 * sub + 64], CPY)
            pending.clear()

        def mm_chunk(quarter, m, n0, xh_box):
            xh = xh_box[0]
            ps = psg.tile([P, 1024], f32, tag="sg", name="ps")[:, :512]
            for c in range(6):
                nc.tensor.matmul(ps, wkv_t[:, c, P * m:P * m + P],
                                 xh[:, c, n0:n0 + 512],
                                 start=(c == 0), stop=(c == 5))
            flush_pending()
            g0 = QT4 * quarter + n0
            has_v = any(kind == "V" for kind, _ in kv_map[m])
            if has_v:
                # full-block bias-add (per-partition bias covers both
                # 64-row subs); K rows are also staged but unused
                vst = vstp.tile([P, 512], f16, tag="vst", name="vst")
                nc.vector.tensor_scalar(vst, ps, b_t[:, m:m + 1], None, ADD)
                pending.append((m, g0, vst))
            for sub, (kind, h) in enumerate(kv_map[m]):
                if kind == "K":
                    rows = slice(64 * sub, 64 * sub + 64)
                    nc.vector.tensor_scalar(
                        kt_t[h][0:64, g0:g0 + 512], ps[rows],
                        b_t[rows, m:m + 1], None, ADD)

        def kt_fixup(b0, nb=8):
            # diag-pack fixup for k-blocks [b0, b0+nb), so the slots that
            # only need kv up to here can start immediately
            flush_pending()
            for h in range(NH):
                ot = kt_t[h][0:64].rearrange(
                    "p (b t) -> p b t", t=P)[:, b0:b0 + nb, 64:P]
                ob = kt_t[h][64:P].rearrange(
                    "p (b t) -> p b t", t=P)[:, b0:b0 + nb, 64:P]
                nc.sync.dma_start(ob, ot)
                nc.vector.memset(ot, 0.0)

        xh_boxes = [[None] for _ in range(4)]

        def make_half_work(quarter, half, with_dma):
            """Half a quarter as a list of small closures, drained one per
            attention group; scheduled just ahead of the first slot whose
            S-groups read these k-blocks."""
            xh_box = xh_boxes[quarter]
            items = []
            if with_dma:
                def dma():
                    xh = xthp.tile([P, 6, QT4], f16, tag="xh", name="xh")
                    nc.sync.dma_start(
                        xh, xt16[:, :, QT4 * quarter:QT4 * (quarter + 1)])
                    xh_box[0] = xh
                items.append(dma)
            n0 = 512 * half
            for m in range(3):
                items.append(
                    lambda m=m: mm_chunk(quarter, m, n0, xh_box))
            items.append(lambda: kt_fixup(8 * quarter + 4 * half, 4))
            return items

        # quarter 0 runs up front (slots 0-1 need it immediately)
        xh0_box = xh_boxes[0]
        xh0 = xthp.tile([P, 6, QT4], f16, tag="xh", name="xh")
        nc.sync.dma_start(xh0, xt16[:, :, 0:QT4])
        xh0_box[0] = xh0
        wq_t = singles.tile([P, 6, 2 * NH * D], f16)
        nc.sync.dma_start(wq_t, wq16[:, :, :])
        xtq_t = singles.tile([P, 6, tq], f16)
        nc.sync.dma_start(xtq_t, xtq16[:, :, :])
        mask_t = singles.tile([P, nslot, 1024], f16)
        nc.sync.dma_start(mask_t, mask[:, :, :])
        wpj_t = singles.tile([P, 2, C], f16)
        nc.sync.dma_start(wpj_t, wpj16[:, :, :])
        # slot 0 only needs k-blocks 0-3: run just the first half of
        # quarter 0 up front, defer everything else into attention drains
        for m in range(3):
            mm_chunk(0, m, 0, xh0_box)
        kt_fixup(0, 4)

        def make_dma(q):
            def dma():
                xh = xthp.tile([P, 6, QT4], f16, tag="xh", name="xh")
                nc.sync.dma_start(xh, xt16[:, :, QT4 * q:QT4 * (q + 1)])
                xh_boxes[q][0] = xh
            return [dma]

        work_q = []     # deferred phase-1 chunks
        late_q = []     # previous slot's normalize/projection chunks

        # ---- Phase 2: per q-slot: attention, then (next Q | recip | proj) ----
        qt_cur = [None] * NH

        def q_head(j, h):
            psq = psm.tile([P, QC], f32, tag="psm", name="psq")
            for c in range(6):
                nc.tensor.matmul(psq, wq_t[:, c, P * h:P * (h + 1)],
                                 xtq_t[:, c, QC * j:QC * (j + 1)],
                                 start=(c == 0), stop=(c == 5))
            qh = qtp.tile([P, QC], f16, tag=f"qt{h}", name=f"qt{h}")
            nc.vector.tensor_scalar(qh, psq, b_t[:, 3 + h:4 + h], None, ADD)
            qt_cur[h] = qh

        def q_block(j):
            for h in range(NH):
                q_head(j, h)

        q_block(0)

        def do_slot(j):
            yacc = psy.tile([65, NH * QC], f32, tag="yacc", name="yacc")
            den3 = rbp.tile([1, NH * QC], f32, tag="den", name="den")
            my_qt = list(qt_cur)

            for h in range(NH):
                yh = yacc[:, QC * h:QC * (h + 1)]

                def s_group(g):
                    sg = psg.tile([P, 1024], f32, tag="sg", name="sg")
                    for i in range(4):
                        kb = 4 * g + i
                        nc.tensor.matmul(sg[:, QC * i:QC * (i + 1)],
                                         kt_t[h][:, P * kb:P * (kb + 1)],
                                         my_qt[h], start=True, stop=True)
                    return sg

                # software pipeline: issue S(g+1) before PV(g) so the PE has
                # work while ACT runs exp(g)
                sg_cur = s_group(0)
                for g in range(j + 1):
                    # drain BEFORE issuing sg_next: a pending kt fixup must
                    # be emitted before any S-group that reads its blocks
                    if work_q:
                        work_q.pop(0)()
                    sg_next = s_group(g + 1) if g < j else None
                    pt = ptp.tile([P, 1024], f16, tag="pt", name="pt")
                    nc.scalar.activation(pt, sg_cur, EXP, scale=0.125)
                    # previous slot's normalize/project chunks drain AFTER
                    # this exp is emitted, so their Ln/Exp land behind it in
                    # the ACT FIFO and the boundary costs no ACT latency
                    if late_q and not (h == 0 and g == 0):
                        late_q.pop(0)()
                    if g == j:
                        nc.vector.tensor_mul(pt, pt, mask_t[:, j])
                    for i in range(4):
                        kb = 4 * g + i
                        nc.tensor.matmul(yh, vp_t[h][:, kb],
                                         pt[:, QC * i:QC * (i + 1)],
                                         start=(g == 0 and i == 0),
                                         stop=(g == j and i == 3))
                    sg_cur = sg_next
                nc.vector.tensor_copy(den3[:, QC * h:QC * (h + 1)], yh[64:65])
            while late_q:
                late_q.pop(0)()

            # stage unnormalized y' to SBUF: frees yacc for the next slot's
            # PV immediately, and lets the whole normalize chain defer
            ystage = ysp.tile([64, NH * QC], f32, tag="ys", name="ys")
            nc.vector.tensor_copy(ystage, yacc[0:64, :])

            if j + 1 < nslot:
                q_block(j + 1)

            # normalized y^T packed into two head-pair tiles for the
            # K=128 paired projection ([y0;y1] and [y2;y2] w/ halved W)
            ytpair = [ytp.tile([P, QC], f16, tag=f"ytp{p}", name=f"ytp{p}")
                      for p in range(2)]

            def norm_chunk(den3=den3, ystage=ystage, ytpair=ytpair):
                # 1/den: ln then exp(-x) on ACT (one shared table set)
                l3 = rbp.tile([1, NH * QC], f32, tag="l3", name="l3")
                nc.scalar.activation(l3, den3, LN)
                r3 = rbp.tile([1, NH * QC], f32r, tag="r3", name="r3")
                nc.scalar.activation(r3, l3, EXP, scale=-1.0)
                for h in range(NH):
                    bc = psm.tile([64, QC], f32, tag="psm", name="bc")
                    nc.tensor.matmul(bc, ones64, r3[:, QC * h:QC * (h + 1)],
                                     start=True, stop=True)
                    rb = rbp.tile([64, QC], f32, tag=f"rb{h}", name=f"rb{h}")
                    nc.vector.tensor_copy(rb, bc)
                    dsts = ([ytpair[h // 2][64 * (h % 2):64 * (h % 2) + 64]]
                            if h < 2 else
                            [ytpair[1][0:64], ytpair[1][64:P]])
                    for dst in dsts:
                        nc.vector.tensor_mul(
                            dst, ystage[:, QC * h:QC * (h + 1)], rb)

            ost = ostp.tile([P, 2, C], f32, tag="ost", name="ost")
            projwork = []
            for qb in range(2):
                for ci, (n0, nw) in enumerate([(0, 512), (512, 256)]):
                    def run(qb=qb, n0=n0, nw=nw, ci=ci, ost=ost,
                            ytpair=ytpair, j=j):
                        pp = psm.tile([P, 512], f32, tag="psm",
                                      name="pp")[:, :nw]
                        for pr in range(2):
                            nc.tensor.matmul(
                                pp, ytpair[pr][:, P * qb:P * (qb + 1)],
                                wpj_t[:, pr, n0:n0 + nw],
                                start=(pr == 0), stop=(pr == 1))
                        nc.vector.tensor_copy(ost[:, qb, n0:n0 + nw], pp)
                        if ci == 1:
                            nc.sync.dma_start(
                                out[QC * j + P * qb:QC * j + P * (qb + 1), :],
                                ost[:, qb])
                    projwork.append(run)
            if j < nslot - 1:
                # normalize+projection drain inside the next slot's
                # attention loop, after its exps are already in the ACT FIFO
                late_q.append(norm_chunk)
                late_q.extend(projwork)
            else:
                norm_chunk()
                for w in projwork:
                    w()

        # interleave: slot j's group g only reads k-blocks 4g..4g+3, so each
        # half-quarter drains through attention groups just ahead of its
        # first reader -- phase 1 rides the PE's exp-wait gaps of slots 0-6
        sched = {
            0: (make_half_work(0, 1, False) + make_dma(1)
                + make_half_work(1, 0, False) + make_dma(2)),
            2: (make_half_work(1, 1, False)
                + make_half_work(2, 0, False) + make_dma(3)),
            4: make_half_work(2, 1, False) + make_half_work(3, 0, False),
            6: make_half_work(3, 1, False),
        }
        for j in range(nslot):
            if j % 2 == 0:
                if j >= 2:
                    while work_q:   # earlier halves must be complete
                        work_q.pop(0)()
                work_q.extend(sched.pop(j, []))
            do_slot(j)
        while work_q:
            work_q.pop(0)()

    split_multi_waits(nc)
    return nc


def make_in_maps(x, W_qkv, b_qkv, W_proj, Tloc=T):
    """Shard the full inputs into the 8 per-core input maps."""
    nslot = Tloc // (2 * QC)
    tq = nslot * QC
    xT = np.ascontiguousarray(x.reshape(Tloc, C).T).astype(np.float32)
    xT16 = xT.astype(np.float16)
    # [C, T] -> [128, 6, T] (c-block-major partitions)
    xt_l = np.ascontiguousarray(xT16.reshape(6, P, Tloc).transpose(1, 0, 2))

    kk = np.arange(P)
    qq = np.arange(QC)
    in_maps = []
    for core in range(8):
        hg, s = core // 2, core % 2
        heads = [3 * hg + i for i in range(NH)]
        wk = [W_qkv[:, C + 64 * h:C + 64 * h + 64] for h in heads]
        wv = [W_qkv[:, 2 * C + 64 * h:2 * C + 64 * h + 64] for h in heads]
        wkv_c = np.concatenate(wk + wv, axis=1)          # [768, 384]
        wq_c = np.concatenate(
            [np.tile(W_qkv[:, 64 * h:64 * h + 64], (1, 2)) for h in heads],
            axis=1)                                      # [768, 384]
        # head-paired projection: [W_h0; W_h1] and [W_h2/2; W_h2/2]
        wpj_c = np.stack([
            np.concatenate([W_proj[64 * heads[0]:64 * heads[0] + 64, :],
                            W_proj[64 * heads[1]:64 * heads[1] + 64, :]], axis=0),
            np.tile(0.5 * W_proj[64 * heads[2]:64 * heads[2] + 64, :], (2, 1)),
        ], axis=1)                                       # [128, 2, 768]
        wkv_16 = np.ascontiguousarray(
            wkv_c.reshape(6, P, 6 * D).transpose(1, 0, 2)).astype(np.float16)
        wq_16 = np.ascontiguousarray(
            wq_c.reshape(6, P, 2 * NH * D).transpose(1, 0, 2)).astype(np.float16)
        wpj_16 = np.ascontiguousarray(wpj_c).astype(np.float16)

        bk = [b_qkv[C + 64 * h:C + 64 * h + 64] for h in heads]
        bv = [b_qkv[2 * C + 64 * h:2 * C + 64 * h + 64] for h in heads]
        bkv_c = np.concatenate(bk + bv)          # [384]
        bias_c = np.zeros((P, 6), np.float32)
        bias_c[:, 0:3] = bkv_c.reshape(3, P).T
        for hi_, h in enumerate(heads):
            bias_c[0:64, 3 + hi_] = b_qkv[64 * h:64 * h + 64]
            bias_c[64:P, 3 + hi_] = b_qkv[64 * h:64 * h + 64]

        qcols = np.concatenate(
            [np.arange(QC * (2 * j + s), QC * (2 * j + s) + QC) for j in range(nslot)])
        xtq_16 = np.ascontiguousarray(
            xT16[:, qcols].reshape(6, P, tq).transpose(1, 0, 2))

        mask_c = np.zeros((nslot, P, 1024), np.float32)
        for j in range(nslot):
            q0 = QC * (2 * j + s)
            for i in range(4):
                k0 = P * (4 * j + i)
                mask_c[j, :, QC * i:QC * (i + 1)] = (
                    (k0 + kk[:, None]) <= (q0 + qq[None, :]))
        mask_l = np.ascontiguousarray(mask_c.transpose(1, 0, 2)).astype(np.float16)

        in_maps.append({
            "xt16": xt_l, "xtq16": xtq_16,
            "wkv16": wkv_16, "wq16": wq_16, "wpj16": wpj_16,
            "bias": bias_c, "mask": mask_l,
        })
    return in_maps


def unshard(results, b_proj, Tloc=T):
    nslot = Tloc // (2 * QC)
    out = np.zeros((Tloc, C), np.float64)
    for core in range(8):
        s = core % 2
        r = results[core]["out"].astype(np.float64)
        for j in range(nslot):
            g0 = QC * (2 * j + s)
            out[g0:g0 + QC] += r[QC * j:QC * (j + 1)]
    out += b_proj.astype(np.float64)
    return out.astype(np.float32).reshape(1, Tloc, C)


_last_result = {}


def kernel(x, mask, W_qkv, b_qkv, W_proj, b_proj):
    from concourse.bass_utils import run_bass_kernel_spmd
    x = np.asarray(x, np.float32)
    W_qkv = np.asarray(W_qkv, np.float32)
    b_qkv = np.asarray(b_qkv, np.float32)
    W_proj = np.asarray(W_proj, np.float32)
    b_proj = np.asarray(b_proj, np.float32)

    if "nc" not in _nc_cache:
        _nc_cache["nc"] = build_nc(T)
    nc = _nc_cache["nc"]
    in_maps = make_in_maps(x, W_qkv, b_qkv, W_proj, T)
    import os
    kwargs = {}
    if os.environ.get("BASS_KERNEL_TRACE"):
        kwargs = dict(trace=True, trace_cores=list(range(8)))
    res = run_bass_kernel_spmd(nc, in_maps, core_ids=list(range(8)), **kwargs)
    _last_result["res"] = res
    return unshard([r for r in res.results], b_proj, T)
